# Initial kernel scaffold
#
"""BalanceLoss (BCE + OHEM top-k negatives) on 8 trn2 NeuronCores.

Strategy
--------
Data-parallel: the 32x1x640x640 inputs are flattened and split into 8 equal
shards (one per core).  Each core computes four partial sums over its shard:

    sw = sum(gt * mask)                      (positive count)
    sn = sum((1 - gt) * mask)                (negative count)
    sa = sum(gt * mask * ln(pred))           (-pos_loss_sum)
    sb = sum((1 - gt) * mask * ln(1 - pred)) (-neg_loss_sum over ALL negatives)

On the host the shards are merged.  The OHEM top-k reduces to the full
negative sum whenever k = min(sn, 3*sw) == sn (all negatives kept), which is
the regime for this data distribution; an exact host fallback handles k < sn.

Per-core kernel: ScalarE does the two Ln passes (Ln(pred), Ln(1-pred) via the
free affine scale=-1/bias=1), VectorE does four fused
tensor_tensor_reduce passes (product + free-dim reduction in one
instruction), everything overlapped with the HBM DMA stream.
"""

import os
import sys

import numpy as np

# ---------------------------------------------------------------- constants
FULL_SHAPE = (32, 1, 640, 640)
TOT = 32 * 640 * 640          # 13_107_200 elements
N_CORES = 8
PER_CORE = TOT // N_CORES     # 1_638_400
P = 128                       # SBUF partitions
W = PER_CORE // P             # 12_800 free-dim elements per partition
NT = 5                        # tiles per core
F = W // NT                   # 2_560 free-dim elements per tile
NEG_RATIO = 3.0
EPS = 1e-6

_CONCOURSE_PATHS = ("/opt/trn_rl_repo", "/root/.axon_site/_ro/trn_rl_repo")


def _ensure_concourse():
    try:
        import concourse.bass  # noqa: F401
    except ImportError:
        for p in _CONCOURSE_PATHS:
            if os.path.isdir(p) and p not in sys.path:
                sys.path.insert(0, p)
        import concourse.bass  # noqa: F401


_NC_CACHE = None


def _build_nc():
    """Build the per-core Bass program (same program on every core)."""
    global _NC_CACHE
    if _NC_CACHE is not None:
        return _NC_CACHE
    _ensure_concourse()
    import concourse.bass as bass
    import concourse.mybir as mybir
    import concourse.tile as tile

    f32 = mybir.dt.float32
    Alu = mybir.AluOpType
    Act = mybir.ActivationFunctionType

    nc = bass.Bass()
    predD = nc.declare_dram_parameter("pred", [P, W], f32, isOutput=False)
    gtD = nc.declare_dram_parameter("gt", [P, W], f32, isOutput=False)
    maskD = nc.declare_dram_parameter("mask", [P, W], f32, isOutput=False)
    # stats columns: [0:NT]=sum(w) [NT:2NT]=sum(n) [2NT:3NT]=sum(w*l1) [3NT:4NT]=sum(n*l2)
    outD = nc.declare_dram_parameter("stats", [P, 4 * NT], f32, isOutput=True)

    with tile.TileContext(nc) as tc:
        with (
            tc.tile_pool(name="io", bufs=3) as io_pool,
            tc.tile_pool(name="tmp", bufs=2) as tmp_pool,
            tc.tile_pool(name="accp", bufs=1) as acc_pool,
        ):
            acc = acc_pool.tile([P, 4 * NT], f32)
            for t in range(NT):
                pred_t = io_pool.tile([P, F], f32, tag="pred")
                nc.sync.dma_start(pred_t[:], predD[:, bass.ts(t, F)])
                gt_t = io_pool.tile([P, F], f32, tag="gt")
                nc.sync.dma_start(gt_t[:], gtD[:, bass.ts(t, F)])
                mask_t = io_pool.tile([P, F], f32, tag="mask")
                nc.sync.dma_start(mask_t[:], maskD[:, bass.ts(t, F)])

                l1 = tmp_pool.tile([P, F], f32, tag="l1")
                nc.scalar.activation(l1[:], pred_t[:], Act.Ln)
                l2 = tmp_pool.tile([P, F], f32, tag="l2")
                nc.scalar.activation(l2[:], pred_t[:], Act.Ln, bias=1.0, scale=-1.0)

                w = tmp_pool.tile([P, F], f32, tag="w")
                nc.vector.tensor_tensor_reduce(
                    out=w[:], in0=gt_t[:], in1=mask_t[:], scale=1.0, scalar=0.0,
                    op0=Alu.mult, op1=Alu.add, accum_out=acc[:, t : t + 1],
                )
                n = tmp_pool.tile([P, F], f32, tag="n")
                nc.vector.tensor_tensor_reduce(
                    out=n[:], in0=mask_t[:], in1=w[:], scale=1.0, scalar=0.0,
                    op0=Alu.subtract, op1=Alu.add, accum_out=acc[:, NT + t : NT + t + 1],
                )
                ja = tmp_pool.tile([P, F], f32, tag="ja")
                nc.vector.tensor_tensor_reduce(
                    out=ja[:], in0=w[:], in1=l1[:], scale=1.0, scalar=0.0,
                    op0=Alu.mult, op1=Alu.add, accum_out=acc[:, 2 * NT + t : 2 * NT + t + 1],
                )
                jb = tmp_pool.tile([P, F], f32, tag="jb")
                nc.vector.tensor_tensor_reduce(
                    out=jb[:], in0=n[:], in1=l2[:], scale=1.0, scalar=0.0,
                    op0=Alu.mult, op1=Alu.add, accum_out=acc[:, 3 * NT + t : 3 * NT + t + 1],
                )
            nc.sync.dma_start(outD[:], acc[:])

    _NC_CACHE = nc
    return nc


def _final_scalar(sw, sn, sa, sb, pred=None, gt=None, mask=None):
    """Host-side merge of the global sums into the balance loss."""
    pos_count = sw
    neg_total = sn
    pos_loss_sum = -sa
    neg_count = min(neg_total, NEG_RATIO * pos_count)
    if neg_count >= neg_total:
        topk_sum = -sb
    else:
        # exact OHEM fallback (never triggered for the shipped distribution):
        # sum of the k hardest negatives, ties split exactly like a sort.
        k = int(neg_count)
        p = np.asarray(pred, dtype=np.float32).ravel()
        g = np.asarray(gt, dtype=np.float32).ravel()
        m = np.asarray(mask, dtype=np.float32).ravel()
        neg_loss = (1.0 - g) * m * (-np.log1p(-p.astype(np.float64)))
        if k <= 0:
            topk_sum = 0.0
        else:
            part = np.partition(neg_loss, neg_loss.size - k)
            topk_sum = float(part[neg_loss.size - k :].sum())
    if neg_count > 0:
        out = (pos_loss_sum + topk_sum) / (pos_count + neg_count + EPS)
    else:
        out = pos_loss_sum / (pos_count + EPS)
    return np.asarray(out, dtype=np.float32).reshape(())


def run_device(pred, gt, mask, trace=False, **run_kwargs):
    """Shard, run the Bass kernel on 8 cores, return (sums, raw results)."""
    _ensure_concourse()
    from concourse.bass_utils import run_bass_kernel_spmd

    nc = _build_nc()
    shards = []
    for a in (pred, gt, mask):
        arr = np.ascontiguousarray(np.asarray(a, dtype=np.float32)).reshape(
            N_CORES, P, W
        )
        shards.append(arr)
    in_maps = [
        {"pred": shards[0][i], "gt": shards[1][i], "mask": shards[2][i]}
        for i in range(N_CORES)
    ]
    res = run_bass_kernel_spmd(nc, in_maps, list(range(N_CORES)), trace=trace,
                               **run_kwargs)
    stats = np.stack([np.asarray(r["stats"], dtype=np.float64) for r in res.results])
    # stats: [cores, P, 4*NT]
    s = stats.sum(axis=(0, 1))
    sw = s[0:NT].sum()
    sn = s[NT : 2 * NT].sum()
    sa = s[2 * NT : 3 * NT].sum()
    sb = s[3 * NT : 4 * NT].sum()
    return (sw, sn, sa, sb), res


def kernel(pred, gt, mask):
    pred = np.asarray(pred, dtype=np.float32)
    gt = np.asarray(gt, dtype=np.float32)
    mask = np.asarray(mask, dtype=np.float32)
    if pred.shape != FULL_SHAPE:
        # defensive pure-host path for non-conforming shapes
        p64 = pred.astype(np.float64)
        sw = float((gt * mask).sum(dtype=np.float64))
        sn = float(((1.0 - gt) * mask).sum(dtype=np.float64))
        sa = float((gt * mask * np.log(p64)).sum())
        sb = float(((1.0 - gt) * mask * np.log1p(-p64)).sum())
        return _final_scalar(sw, sn, sa, sb, pred, gt, mask)
    (sw, sn, sa, sb), _ = run_device(pred, gt, mask)
    return _final_scalar(sw, sn, sa, sb, pred, gt, mask)


# revision 5
# speedup vs baseline: 27.0182x; 27.0182x over previous
"""BalanceLoss (BCE + OHEM top-k negatives) on 8 trn2 NeuronCores.

Strategy
--------
Data-parallel: the 32x1x640x640 inputs are flattened and split into 8 equal
shards (one per core).  Each core computes four partial sums over its shard:

    sw = sum(gt * mask)                      (positive count)
    sn = sum((1 - gt) * mask)                (negative count)
    sa = sum(gt * mask * ln(pred))           (-pos_loss_sum)
    sb = sum((1 - gt) * mask * ln(1 - pred)) (-neg_loss_sum over ALL negatives)

On the host the shards are merged.  The OHEM top-k reduces to the full
negative sum whenever k = min(sn, 3*sw) == sn (all negatives kept), which is
the regime for this data distribution; an exact host fallback handles k < sn.

Per-core kernel: ScalarE does the two Ln passes (Ln(pred), Ln(1-pred) via the
free affine scale=-1/bias=1), VectorE does four fused
tensor_tensor_reduce passes (product + free-dim reduction in one
instruction), everything overlapped with the HBM DMA stream.
"""

import os
import sys

import numpy as np

# ---------------------------------------------------------------- constants
FULL_SHAPE = (32, 1, 640, 640)
TOT = 32 * 640 * 640          # 13_107_200 elements
N_CORES = 8
PER_CORE = TOT // N_CORES     # 1_638_400
P = 128                       # SBUF partitions
W = PER_CORE // P             # 12_800 free-dim elements per partition
NT = 5                        # tiles per core
F = W // NT                   # 2_560 free-dim elements per tile
NEG_RATIO = 3.0
EPS = 1e-6

_CONCOURSE_PATHS = ("/opt/trn_rl_repo", "/root/.axon_site/_ro/trn_rl_repo")


def _ensure_concourse():
    try:
        import concourse.bass  # noqa: F401
    except ImportError:
        for p in _CONCOURSE_PATHS:
            if os.path.isdir(p) and p not in sys.path:
                sys.path.insert(0, p)
        import concourse.bass  # noqa: F401


_NC_CACHE = {}


def _build_nc(reps=1):
    """Build the per-core Bass program (same program on every core).

    reps > 1 unrolls the whole tile loop `reps` times inside one NEFF for
    benchmarking; the fused accum writes are idempotent so results are
    unchanged."""
    if reps in _NC_CACHE:
        return _NC_CACHE[reps]
    _ensure_concourse()
    import concourse.bacc as bacc
    import concourse.bass as bass
    import concourse.mybir as mybir
    import concourse.tile as tile

    f32 = mybir.dt.float32
    Act = mybir.ActivationFunctionType

    nc = bacc.Bacc(None, target_bir_lowering=False)
    predD = nc.declare_dram_parameter("pred", [P, W], f32, isOutput=False)
    gtD = nc.declare_dram_parameter("gt", [P, W], f32, isOutput=False)
    maskD = nc.declare_dram_parameter("mask", [P, W], f32, isOutput=False)
    # stats columns: [0:NT]=sum(w) [NT:2NT]=sum(n) [2NT:3NT]=sum(w*l1) [3NT:4NT]=sum(n*l2)
    outD = nc.declare_dram_parameter("stats", [P, 4 * NT], f32, isOutput=True)

    with tile.TileContext(nc) as tc:
        with (
            tc.tile_pool(name="io", bufs=3) as io_pool,
            tc.tile_pool(name="tmp", bufs=2) as tmp_pool,
            tc.tile_pool(name="accp", bufs=1) as acc_pool,
        ):
            acc = acc_pool.tile([P, 4 * NT], f32)
            for t in [t for _ in range(reps) for t in range(NT)]:
                pred_t = io_pool.tile([P, F], f32, tag="pred")
                nc.sync.dma_start(pred_t[:], predD[:, bass.ts(t, F)])
                gt_t = io_pool.tile([P, F], f32, tag="gt")
                nc.sync.dma_start(gt_t[:], gtD[:, bass.ts(t, F)])
                mask_t = io_pool.tile([P, F], f32, tag="mask")
                nc.sync.dma_start(mask_t[:], maskD[:, bass.ts(t, F)])

                l1 = tmp_pool.tile([P, F], f32, tag="l1")
                nc.scalar.activation(l1[:], pred_t[:], Act.Ln)
                l2 = tmp_pool.tile([P, F], f32, tag="l2")
                nc.scalar.activation(l2[:], pred_t[:], Act.Ln, bias=1.0, scale=-1.0)

                # w = gt*mask, n = (1-gt)*mask, each with fused free-dim sum
                w = tmp_pool.tile([P, F], f32, tag="w")
                nc.vector.affine_mul_reduce(
                    out=w[:], accum_out=acc[:, t : t + 1],
                    in0=gt_t[:], in1=mask_t[:], scale=1.0, bias=0.0,
                )
                n = tmp_pool.tile([P, F], f32, tag="n")
                nc.vector.affine_mul_reduce(
                    out=n[:], accum_out=acc[:, NT + t : NT + t + 1],
                    in0=gt_t[:], in1=mask_t[:], scale=-1.0, bias=1.0,
                )
                ja = tmp_pool.tile([P, F], f32, tag="junk")
                nc.vector.affine_mul_reduce(
                    out=ja[:], accum_out=acc[:, 2 * NT + t : 2 * NT + t + 1],
                    in0=w[:], in1=l1[:], scale=1.0, bias=0.0,
                )
                jb = tmp_pool.tile([P, F], f32, tag="junk")
                nc.vector.affine_mul_reduce(
                    out=jb[:], accum_out=acc[:, 3 * NT + t : 3 * NT + t + 1],
                    in0=n[:], in1=l2[:], scale=1.0, bias=0.0,
                )
            nc.sync.dma_start(outD[:], acc[:])
    nc.finalize()

    _NC_CACHE[reps] = nc
    return nc


def _final_scalar(sw, sn, sa, sb, pred=None, gt=None, mask=None):
    """Host-side merge of the global sums into the balance loss."""
    pos_count = sw
    neg_total = sn
    pos_loss_sum = -sa
    neg_count = min(neg_total, NEG_RATIO * pos_count)
    if neg_count >= neg_total:
        topk_sum = -sb
    else:
        # exact OHEM fallback (never triggered for the shipped distribution):
        # sum of the k hardest negatives, ties split exactly like a sort.
        k = int(neg_count)
        p = np.asarray(pred, dtype=np.float32).ravel()
        g = np.asarray(gt, dtype=np.float32).ravel()
        m = np.asarray(mask, dtype=np.float32).ravel()
        neg_loss = (1.0 - g) * m * (-np.log1p(-p.astype(np.float64)))
        if k <= 0:
            topk_sum = 0.0
        else:
            part = np.partition(neg_loss, neg_loss.size - k)
            topk_sum = float(part[neg_loss.size - k :].sum())
    if neg_count > 0:
        out = (pos_loss_sum + topk_sum) / (pos_count + neg_count + EPS)
    else:
        out = pos_loss_sum / (pos_count + EPS)
    return np.asarray(out, dtype=np.float32).reshape(())


def run_device(pred, gt, mask, trace=False, reps=1, **run_kwargs):
    """Shard, run the Bass kernel on 8 cores, return (sums, raw results)."""
    _ensure_concourse()
    from concourse.bass_utils import run_bass_kernel_spmd

    nc = _build_nc(reps)
    shards = []
    for a in (pred, gt, mask):
        arr = np.ascontiguousarray(np.asarray(a, dtype=np.float32)).reshape(
            N_CORES, P, W
        )
        shards.append(arr)
    in_maps = [
        {"pred": shards[0][i], "gt": shards[1][i], "mask": shards[2][i]}
        for i in range(N_CORES)
    ]
    res = run_bass_kernel_spmd(nc, in_maps, list(range(N_CORES)), trace=trace,
                               **run_kwargs)
    stats = np.stack([np.asarray(r["stats"], dtype=np.float64) for r in res.results])
    # stats: [cores, P, 4*NT]
    s = stats.sum(axis=(0, 1))
    sw = s[0:NT].sum()
    sn = s[NT : 2 * NT].sum()
    sa = s[2 * NT : 3 * NT].sum()
    sb = s[3 * NT : 4 * NT].sum()
    return (sw, sn, sa, sb), res


def kernel(pred, gt, mask):
    pred = np.asarray(pred, dtype=np.float32)
    gt = np.asarray(gt, dtype=np.float32)
    mask = np.asarray(mask, dtype=np.float32)
    if pred.shape != FULL_SHAPE:
        # defensive pure-host path for non-conforming shapes
        p64 = pred.astype(np.float64)
        sw = float((gt * mask).sum(dtype=np.float64))
        sn = float(((1.0 - gt) * mask).sum(dtype=np.float64))
        sa = float((gt * mask * np.log(p64)).sum())
        sb = float(((1.0 - gt) * mask * np.log1p(-p64)).sum())
        return _final_scalar(sw, sn, sa, sb, pred, gt, mask)
    (sw, sn, sa, sb), _ = run_device(pred, gt, mask)
    return _final_scalar(sw, sn, sa, sb, pred, gt, mask)
